# revision 43
# baseline (speedup 1.0000x reference)
"""Multi-head attention (B=2, N=2048, C=1024, H=16, D=64) on 8 TRN2 NeuronCores.

Sharding: core c = (batch b = c//4) x (head-group g = c%4 -> heads 4g..4g+3).
Data parallel on B, tensor parallel on heads.  Each core emits its
out-projection PARTIAL [C, N] in bf16; the host sums the 4 partials of each
batch group and adds the (folded) bias.  No on-device collectives.

Softmax exp is split across two engines: the scalar engine computes true
exp on one 512-column half of each [128, 1024] score tile while the vector
engine computes a Schraudolph bit-trick exp (uint16 = round(A*raw + B),
bits reinterpreted as bf16) on the other half; halves swap each j so the
~3% approx error mixes across queries.

Everything on device stays transposed ([channel, position]); the host
pre-transposes inputs and post-transposes the output.
"""

import numpy as np
import ml_dtypes

import concourse.bacc as bacc
import concourse.tile as tile
import concourse.mybir as mybir
from concourse.bass_utils import run_bass_kernel_spmd

B, N, C, H = 2, 2048, 1024, 16
D = C // H          # 64
HL = H // 4         # 4 heads per core
CL = HL * D         # 256 local channels
N_CORES = 8

F32 = mybir.dt.float32
F16 = mybir.dt.float16
BF16 = mybir.dt.bfloat16
U16 = mybir.dt.uint16
NPF16 = np.float16
NPBF16 = ml_dtypes.bfloat16

KC = C // 128       # 8  K-chunks of the input channel dim
NJ = N // 128       # 16 128-row j-chunks
NQ = N // 1024      # 2  1024-wide column blocks for the qk projection

SCALE = float(1.0 / np.sqrt(D))
SHIFT = -16.0       # static exp shift; softmax is shift-invariant
# Schraudolph fake-exp in the bf16 bit domain:
#   u16 = round(EXP_A * raw + EXP_B); bits(u16 << 16) ~ exp(SCALE*raw + SHIFT)
_S = 128.0 / float(np.log(2.0))
EXP_A = float(_S * SCALE)
EXP_B = float(_S * SHIFT + 127.0 * 128.0 - 4.75)


def build_kernel(n_cores=N_CORES):
    nc = bacc.Bacc("TRN2", target_bir_lowering=False, debug=False,
                   num_devices=n_cores)

    xT = nc.declare_dram_parameter("xT", [C, N], F16, isOutput=False)
    cos2 = nc.declare_dram_parameter("cos2", [128, N], F16, isOutput=False)
    sin2s = nc.declare_dram_parameter("sin2s", [128, N], F16, isOutput=False)
    wqkT = nc.declare_dram_parameter("wqkT", [C, 2 * CL], F16, isOutput=False)
    bqk = nc.declare_dram_parameter("bqk", [2 * CL, 1], F32, isOutput=False)
    wvT = nc.declare_dram_parameter("wvT", [C, CL], F16, isOutput=False)
    wprojT = nc.declare_dram_parameter("wprojT", [CL, C], BF16, isOutput=False)
    outp = nc.declare_dram_parameter("outp", [C, N], BF16, isOutput=True)

    with tile.TileContext(nc) as tc:
        with tc.tile_pool(name="sbuf", bufs=1) as sb, \
             tc.tile_pool(name="psum", bufs=1, space="PSUM") as ps:

            # tile for clock-warming matmuls (see _warm_pe)
            warm = sb.tile([128, 128], F16, name="warm", tag="warm")
            nc.vector.memset(warm[:], 0.001)

            def _warm_pe(tag, n):
                # short matmuls alternating two PSUM tiles: keeps the PE's
                # activity monitor busy so the clock gate stays at full rate
                # (~60ns each warm; size n to span the bridge window)
                wps = [ps.tile([128, 64], F32, name=f"warmp{tag}_{a}",
                               tag="sc", bufs=2) for a in range(2)]
                for r in range(n):
                    nc.tensor.matmul(wps[r % 2][:], warm[:], warm[:, :64],
                                     start=True, stop=True)

            # run a warm burst during the input-DMA dead window so the qk
            # projection starts with the clock gate already released
            _warm_pe("s", 40)

            # ---- load inputs ----
            # x lands as column blocks (all channel rows of positions
            # [1024*nq, 1024*(nq+1))) so the nq-outer qk projection can start
            # after ~2MB instead of the full 4MB transfer
            xb = [[None] * KC for _ in range(NQ)]
            wqk_sb = []
            for kc in range(KC):
                t = sb.tile([128, 2 * CL], F16, name=f"wqk{kc}", tag=f"wqk{kc}")
                eng = nc.scalar if kc % 2 == 0 else nc.sync
                eng.dma_start(t[:], wqkT.ap()[128 * kc:128 * (kc + 1), :])
                wqk_sb.append(t)
                t = sb.tile([128, 1024], F16, name=f"xb0_{kc}", tag=f"xb0_{kc}")
                eng = nc.sync if kc % 2 == 0 else nc.scalar
                eng.dma_start(t[:], xT.ap()[128 * kc:128 * (kc + 1), 0:1024])
                xb[0][kc] = t
            bqk_sb = []
            for m in range(4):
                t = sb.tile([128, 1], F32, name=f"bqk{m}", tag=f"bqk{m}")
                nc.sync.dma_start(t[:], bqk.ap()[128 * m:128 * (m + 1), :])
                bqk_sb.append(t)
            for kc in range(KC):
                t = sb.tile([128, 1024], F16, name=f"xb1_{kc}", tag=f"xb1_{kc}")
                eng = nc.sync if kc % 2 == 0 else nc.scalar
                eng.dma_start(t[:], xT.ap()[128 * kc:128 * (kc + 1), 1024:2048])
                xb[1][kc] = t
            cos_sb = sb.tile([128, N], F16, name="cos_sb", tag="cos_sb")
            nc.sync.dma_start(cos_sb[:], cos2.ap())
            sin_sb = sb.tile([128, N], F16, name="sin_sb", tag="sin_sb")
            nc.scalar.dma_start(sin_sb[:], sin2s.ap())
            wv_sb = []
            for kc in range(KC):
                t = sb.tile([128, CL], F16, name=f"wv{kc}", tag=f"wv{kc}")
                eng = nc.sync if kc % 2 == 0 else nc.scalar
                eng.dma_start(t[:], wvT.ap()[128 * kc:128 * (kc + 1), :])
                wv_sb.append(t)
            wproj_sb = []
            for p in range(2):
                t = sb.tile([128, C], BF16, name=f"wproj{p}", tag=f"wproj{p}")
                nc.sync.dma_start(t[:], wprojT.ap()[128 * p:128 * (p + 1), :])
                wproj_sb.append(t)

            # ---- qk projection + RoPE ----
            # chunk m rows: m=0:[q_h0,q_h1] m=1:[q_h2,q_h3] m=2:[k_h0,k_h1] m=3:[k_h2,k_h3]
            # Per-head q/k land DUPLICATED into both 64-partition halves of
            # their own [128, N] tile (second copy via SBUF->SBUF DMA on the
            # idle sync/gpsimd queues): the scores matmul then runs as TWO
            # CONCURRENT K=64 row-tiled matmuls (tile_position (0,0)/(64,0)),
            # one per 512-query half -- half the PE time of the zero-padded
            # K=128 form.
            q_dup = [sb.tile([128, N], F16, name=f"qdup{h}", tag=f"qdup{h}")
                     for h in range(4)]
            k_dup = [sb.tile([128, N], F16, name=f"kdup{h}", tag=f"kdup{h}")
                     for h in range(4)]
            swap_mask = [i ^ 1 for i in range(32)]
            qks_t = [sb.tile([128, N], F16, name=f"qks{m}", tag=f"qks{m}")
                     for m in range(4)]
            _dup_eng = [0]

            def rope_slice(m, nq):
                # RoPE on a [128, 1024] column slice of qks[m]:
                #   qk' = qks*cos2 + shift(qks)*sin2s
                # (pair-swap of adjacent partitions via DVE stream shuffle)
                sl = slice(1024 * nq, 1024 * (nq + 1))
                qks = qks_t[m]
                shf = sb.tile([128, 1024], F16, name=f"shf{m}_{nq}", tag="shf",
                              bufs=2)
                nc.vector.stream_shuffle(shf[:], qks[:, sl], swap_mask)
                t2 = sb.tile([128, 1024], F16, name=f"rtmp{m}_{nq}",
                             tag="ropetmp", bufs=2)
                nc.vector.tensor_mul(t2[:], shf[:], sin_sb[:, sl])
                t1 = sb.tile([128, 1024], F16, name=f"rtc{m}_{nq}",
                             tag="ropetc", bufs=2)
                nc.vector.tensor_mul(t1[:], qks[:, sl], cos_sb[:, sl])
                dup = q_dup if m < 2 else k_dup
                lo, hi = dup[2 * (m % 2)], dup[2 * (m % 2) + 1]
                nc.vector.tensor_add(lo[0:64, sl], t1[0:64, :], t2[0:64, :])
                nc.vector.tensor_add(hi[64:128, sl], t1[64:128, :],
                                     t2[64:128, :])
                for t, src, dst in ((lo, slice(0, 64), slice(64, 128)),
                                    (hi, slice(64, 128), slice(0, 64))):
                    eng = nc.sync if _dup_eng[0] % 2 == 0 else nc.gpsimd
                    _dup_eng[0] += 1
                    eng.dma_start(t[dst, sl], t[src, sl])

            # nq-outer so the first matmuls need only the first x column
            # block; 4 m-accumulators fill all 8 PSUM banks per nq.
            # Evict order [2,0,3,1] + selective RoPE so heads 0/1 (k from
            # m=2, q from m=0) are ready as early as possible; the remaining
            # RoPE slices are interleaved into head 0's attention pipeline
            # where the vector engine has slack.
            for nq in range(NQ):
                accs = [ps.tile([128, 1024], F32, name=f"qacc{nq}_{m}",
                                tag="sc" if m < 2 else "oacc", bufs=2)
                        for m in range(4)]
                for kc in range(KC):
                    for m in range(4):
                        for n2 in range(2):
                            nc.tensor.matmul(
                                accs[m][:, 512 * n2:512 * (n2 + 1)],
                                wqk_sb[kc][:, 128 * m:128 * (m + 1)],
                                xb[nq][kc][:, 512 * n2:512 * (n2 + 1)],
                                start=(kc == 0), stop=(kc == KC - 1))
                for m in (2, 0, 3, 1):
                    nc.scalar.activation(
                        qks_t[m][:, 1024 * nq:1024 * (nq + 1)],
                        accs[m][:],
                        mybir.ActivationFunctionType.Identity,
                        bias=bqk_sb[m][:])
                if nq == 0:
                    rope_slice(2, 0)
                    rope_slice(0, 0)
                else:
                    rope_slice(2, 1)
            rope_deferred = [(1, 0), (3, 0), (3, 1), (0, 1), (1, 1)]

            # ---- v projection (natural [j, ch] layout, ones col appended per head) ----
            # j-chunk pairs; pairs 0-1 run before attention, pairs 2-7 are
            # interleaved into head 0's matmul stream (the exp engines set
            # head 0's pace, so the PE slots are free)
            vaug = [None] * NJ

            def vproj_pair(jp):
                jcs = (2 * jp, 2 * jp + 1)
                pvs = [ps.tile([128, CL], F32, name=f"pv{jc}", tag="sc",
                               bufs=2) for jc in jcs]
                for kc in range(KC):
                    for a, jc in enumerate(jcs):
                        nc.tensor.matmul(
                            pvs[a][:],
                            xb[jc // 8][kc][:, 128 * (jc % 8):128 * (jc % 8 + 1)],
                            wv_sb[kc][:],
                            start=(kc == 0), stop=(kc == KC - 1))
                for a, jc in enumerate(jcs):
                    va = sb.tile([128, HL * (D + 1)], BF16, name=f"vaug{jc}",
                                 tag=f"vaug{jc}")
                    nc.vector.memset(va[:, D::D + 1], 1.0)
                    nc.scalar.activation(
                        va.rearrange("p (h e) -> p h e", e=D + 1)[:, :, 0:D],
                        pvs[a].rearrange("p (h e) -> p h e", e=D)[:, :, :],
                        mybir.ActivationFunctionType.Copy)
                    vaug[jc] = va

            for jp in range(NJ // 2):
                vproj_pair(jp)
            for mq in rope_deferred:
                rope_slice(*mq)

            # per-partition bias AP used to shift scores before exp
            eshift = sb.tile([128, 1], F32, name="eshift", tag="eshift")
            nc.vector.memset(eshift[:], SHIFT)
            # K=1 ones row used to broadcast denominators across partitions
            ones64 = sb.tile([1, 64], BF16, name="ones64", tag="ones64")
            nc.vector.memset(ones64[:], 1.0)

            # ---- attention + projection partials, per i-chunk ----
            chunks = [(0, 1024), (1024, 1024)]

            def finalize_head(ih, hl, oacc, o_pair, cw):
                # normalize: o[:, i] / den[i].  Broadcast den across
                # partitions with a K=1 matmul, then reciprocal+mul.
                den = sb.tile([1, cw], BF16, name=f"den{ih}_{hl}",
                              tag="den", bufs=2)
                # vector engine: the scalar queue is the attention j-limiter
                # and an extra ~1.1us ACTIVATE per head makes it slip
                nc.vector.tensor_copy(den[:], oacc[64:65, :])
                rb = ps.tile([64, cw], F32, name=f"rb{ih}_{hl}",
                             tag="oacc", bufs=2)
                for q in range(cw // 512):
                    nc.tensor.matmul(rb[:, 512 * q:512 * (q + 1)],
                                     ones64[:],
                                     den[:, 512 * q:512 * (q + 1)],
                                     start=True, stop=True)
                rr = sb.tile([64, cw], F32, name=f"rr{ih}_{hl}", tag="rr",
                             bufs=2)
                nc.vector.reciprocal_approx_fast(rr[:], rb[:])
                nc.vector.tensor_mul(
                    o_pair[hl // 2][64 * (hl % 2):64 * (hl % 2) + 64, :],
                    oacc[0:64, :], rr[:])

            for ih, (i0, cw) in enumerate(chunks):
                ns = cw // 512
                o_pair = [sb.tile([128, cw], BF16, name=f"opair{ih}_{p}",
                                  tag=f"opair{p}", bufs=2) for p in range(2)]
                if ih > 0:
                    _warm_pe(f"c{ih}", 8)
                pending = None
                pending_o = None
                for hl in range(4):
                    qT = q_dup[hl]
                    kT = k_dup[hl]
                    oacc = ps.tile([65, cw], F32, name=f"oacc{ih}_{hl}",
                                   tag="oacc", bufs=2)
                    exs = []

                    def emit_o(jc, oacc=oacc, exs=exs, hl=hl):
                        for q in range(ns):
                            nc.tensor.matmul(
                                oacc[:, 512 * q:512 * (q + 1)],
                                vaug[jc][:, (D + 1) * hl:(D + 1) * (hl + 1)],
                                exs[jc][:, 512 * q:512 * (q + 1)],
                                start=(jc == 0), stop=(jc == NJ - 1))

                    for jc in range(NJ):
                        sc = ps.tile([128, cw], F32, name=f"sc{ih}_{hl}_{jc}",
                                     tag="sc", bufs=2)
                        # two concurrent K=64 row-tiled matmuls, one per
                        # 512-query half (disjoint row-groups 0-1 / 2-3)
                        for q in range(ns):
                            rp = 64 * (q % 2)
                            nc.tensor.matmul(
                                sc[:, 512 * q:512 * (q + 1)],
                                kT[rp:rp + 64, 128 * jc:128 * (jc + 1)],
                                qT[rp:rp + 64, i0 + 512 * q:i0 + 512 * (q + 1)],
                                start=True, stop=True,
                                tile_position=(rp, 0))
                        ex = sb.tile([128, cw], BF16, name=f"ex{ih}_{hl}_{jc}",
                                     tag="ex", bufs=6)
                        # exp split: scalar engine takes a 576-col stretch
                        # (true exp), vector engine the other 448 (bit-trick
                        # exp; DVE reads PSUM at ~half rate so it gets the
                        # smaller share); sides swap each j to mix the
                        # approx error across queries
                        wa = 640
                        pa = (jc % 2) * (cw - wa)
                        pv = (wa if jc % 2 == 0 else 0)
                        nc.scalar.activation(ex[:, pa:pa + wa],
                                             sc[:, pa:pa + wa],
                                             mybir.ActivationFunctionType.Exp,
                                             scale=SCALE,
                                             bias=eshift[:])
                        nc.vector.tensor_scalar(
                            ex[:, pv:pv + (cw - wa)].bitcast(U16),
                            sc[:, pv:pv + (cw - wa)],
                            EXP_A, EXP_B,
                            mybir.AluOpType.mult, mybir.AluOpType.add)
                        exs.append(ex)
                        # software pipeline: o-matmuls lag TWO j-chunks so
                        # the ~1.2us scores->exp->weights latency stays off
                        # the matmul critical path; the previous head's LAST
                        # two o-matmuls and its normalization chain are both
                        # deferred into this head's pipeline so neither the
                        # matmul nor the exp stream pauses at head boundaries
                        if jc == 1:
                            if pending_o is not None:
                                for t in range(NJ - 2, NJ):
                                    pending_o(t)
                                pending_o = None
                            if pending is not None:
                                finalize_head(*pending)
                                pending = None
                        if jc >= 2:
                            emit_o(jc - 2)
                    pending_o = emit_o
                    pending = (ih, hl, oacc, o_pair, cw)
                # flush the last head's deferred o-matmuls + normalization;
                # a short warm burst bridges the PE through the chain so the
                # projection starts at full rate
                if pending_o is not None:
                    for t in range(NJ - 2, NJ):
                        pending_o(t)
                    pending_o = None
                _warm_pe(f"p{ih}", 12)
                finalize_head(*pending)
                pending = None

                # out-projection partial for this i-chunk, written straight
                # to the output (host sums the 4 per-group partials)
                for mc in range(8):
                    pp = ps.tile([128, cw], F32, name=f"pp{ih}_{mc}",
                                 tag="sc" if mc % 2 == 0 else "oacc",
                                 bufs=2)
                    for n2 in range(ns):
                        isl = slice(512 * n2, 512 * (n2 + 1))
                        for p in range(2):
                            nc.tensor.matmul(
                                pp[:, isl],
                                wproj_sb[p][:, 128 * mc:128 * (mc + 1)],
                                o_pair[p][:, isl],
                                start=(p == 0), stop=(p == 1))
                    po = sb.tile([128, cw], BF16, name=f"po{ih}_{mc}",
                                 tag="po", bufs=4)
                    # alternate evict engine so slots recycle 2x faster
                    if mc % 2 == 0:
                        nc.vector.tensor_copy(po[:], pp[:])
                    else:
                        nc.scalar.activation(
                            po[:], pp[:],
                            mybir.ActivationFunctionType.Copy)
                    eng = nc.sync if mc % 2 == 0 else nc.gpsimd
                    eng.dma_start(
                        outp.ap()[128 * mc:128 * (mc + 1), i0:i0 + cw], po[:])

    nc.compile()
    return nc


def shard_inputs(x, rope, w_qkv, b_qkv, w_proj, b_proj, n_cores=N_CORES):
    """Per-core input maps. Host-side transposes/casts are part of sharding."""
    in_maps = []
    for c in range(n_cores):
        b = (c // 4) % B
        g = c % 4
        heads = range(HL * g, HL * g + HL)

        xTb = np.ascontiguousarray(x[b].T).astype(NPF16)          # [C, N]

        cosT = rope[b].T[:D, :]                                   # [64, N]
        sinT = rope[b].T[D:, :]
        cos2 = np.vstack([cosT, cosT]).astype(NPF16)              # [128, N]
        sgn = np.where(np.arange(128) % 2 == 0, -1.0, 1.0)[:, None]
        sin2s = (np.vstack([sinT, sinT]) * sgn).astype(NPF16)     # [128, N]

        # qk weight rows ordered [q_h0..q_h3, k_h0..k_h3]
        qk_rows = []
        bqk_rows = []
        for h in heads:
            qk_rows.append(w_qkv[D * h:D * (h + 1), :])           # q rows
            bqk_rows.append(b_qkv[D * h:D * (h + 1)])
        for h in heads:
            qk_rows.append(w_qkv[C + D * h:C + D * (h + 1), :])   # k rows
            bqk_rows.append(b_qkv[C + D * h:C + D * (h + 1)])
        wqk = np.vstack(qk_rows)                                  # [512, C]
        wqkT = np.ascontiguousarray(wqk.T).astype(NPF16)          # [C, 512]
        bqk_v = np.concatenate(bqk_rows).astype(np.float32)[:, None]

        h0 = HL * g
        wv = w_qkv[2 * C + D * h0:2 * C + D * h0 + CL, :]          # [256, C]
        wvT = np.ascontiguousarray(wv.T).astype(NPF16)             # [C, 256]

        wp = w_proj[:, D * h0:D * h0 + CL]                         # [C, 256]
        wprojT = np.ascontiguousarray(wp.T).astype(NPBF16)         # [256, C]

        in_maps.append({
            "xT": xTb, "cos2": cos2, "sin2s": sin2s,
            "wqkT": wqkT, "bqk": bqk_v, "wvT": wvT,
            "wprojT": wprojT,
        })
    return in_maps


def assemble(results, b_eff, n_cores=N_CORES):
    out = np.empty((B, N, C), dtype=np.float32)
    for b in range(B):
        acc = np.zeros((C, N), dtype=np.float32)
        for g in range(4):
            acc += results[4 * b + g]["outp"].astype(np.float32)
        out[b] = acc.T + b_eff[None, :]
    return out


_NC_CACHE = {}


def _get_nc():
    if "nc" not in _NC_CACHE:
        _NC_CACHE["nc"] = build_kernel()
    return _NC_CACHE["nc"]


def _run(inputs, trace=False, tmpdir=None):
    nc = _get_nc()
    inputs = {k: np.asarray(v) for k, v in inputs.items()}
    # fold the v-bias through the projection into the output bias (host side)
    b_v = inputs["b_qkv"][2 * C:3 * C]
    b_eff = (inputs["b_proj"] + b_v @ inputs["w_proj"].T).astype(np.float32)
    in_maps = shard_inputs(**inputs)
    res = run_bass_kernel_spmd(nc, in_maps, core_ids=list(range(N_CORES)),
                               trace=trace, tmpdir=tmpdir)
    return assemble(res.results, b_eff), res


def kernel(**inputs):
    out, _ = _run(inputs)
    return out


# revision 44
# speedup vs baseline: 1.0067x; 1.0067x over previous
"""Multi-head attention (B=2, N=2048, C=1024, H=16, D=64) on 8 TRN2 NeuronCores.

Sharding: core c = (batch b = c//4) x (head-group g = c%4 -> heads 4g..4g+3).
Data parallel on B, tensor parallel on heads.  Each core emits its
out-projection PARTIAL [C, N] in bf16; the host sums the 4 partials of each
batch group and adds the (folded) bias.  No on-device collectives.

Softmax exp is split across two engines: the scalar engine computes true
exp on one 512-column half of each [128, 1024] score tile while the vector
engine computes a Schraudolph bit-trick exp (uint16 = round(A*raw + B),
bits reinterpreted as bf16) on the other half; halves swap each j so the
~3% approx error mixes across queries.

Everything on device stays transposed ([channel, position]); the host
pre-transposes inputs and post-transposes the output.
"""

import numpy as np
import ml_dtypes

import concourse.bacc as bacc
import concourse.tile as tile
import concourse.mybir as mybir
from concourse.bass_utils import run_bass_kernel_spmd

B, N, C, H = 2, 2048, 1024, 16
D = C // H          # 64
HL = H // 4         # 4 heads per core
CL = HL * D         # 256 local channels
N_CORES = 8

F32 = mybir.dt.float32
F16 = mybir.dt.float16
BF16 = mybir.dt.bfloat16
U16 = mybir.dt.uint16
NPF16 = np.float16
NPBF16 = ml_dtypes.bfloat16

KC = C // 128       # 8  K-chunks of the input channel dim
NJ = N // 128       # 16 128-row j-chunks
NQ = N // 1024      # 2  1024-wide column blocks for the qk projection

SCALE = float(1.0 / np.sqrt(D))
SHIFT = -16.0       # static exp shift; softmax is shift-invariant
# Schraudolph fake-exp in the bf16 bit domain:
#   u16 = round(EXP_A * raw + EXP_B); bits(u16 << 16) ~ exp(SCALE*raw + SHIFT)
_S = 128.0 / float(np.log(2.0))
EXP_A = float(_S * SCALE)
EXP_B = float(_S * SHIFT + 127.0 * 128.0 - 4.75)


def build_kernel(n_cores=N_CORES):
    nc = bacc.Bacc("TRN2", target_bir_lowering=False, debug=False,
                   num_devices=n_cores)

    xT = nc.declare_dram_parameter("xT", [C, N], F16, isOutput=False)
    cos2 = nc.declare_dram_parameter("cos2", [128, N], F16, isOutput=False)
    sin2s = nc.declare_dram_parameter("sin2s", [128, N], F16, isOutput=False)
    wqkT = nc.declare_dram_parameter("wqkT", [C, 2 * CL], F16, isOutput=False)
    bqk = nc.declare_dram_parameter("bqk", [2 * CL, 1], F32, isOutput=False)
    wvT = nc.declare_dram_parameter("wvT", [C, CL], F16, isOutput=False)
    wprojT = nc.declare_dram_parameter("wprojT", [CL, C], BF16, isOutput=False)
    outp = nc.declare_dram_parameter("outp", [C, N], BF16, isOutput=True)

    with tile.TileContext(nc) as tc:
        with tc.tile_pool(name="sbuf", bufs=1) as sb, \
             tc.tile_pool(name="psum", bufs=1, space="PSUM") as ps:

            # tile for clock-warming matmuls (see _warm_pe)
            warm = sb.tile([128, 128], F16, name="warm", tag="warm")
            nc.vector.memset(warm[:], 0.001)

            def _warm_pe(tag, n):
                # short matmuls alternating two PSUM tiles: keeps the PE's
                # activity monitor busy so the clock gate stays at full rate
                # (~60ns each warm; size n to span the bridge window)
                wps = [ps.tile([128, 64], F32, name=f"warmp{tag}_{a}",
                               tag="sc", bufs=2) for a in range(2)]
                for r in range(n):
                    nc.tensor.matmul(wps[r % 2][:], warm[:], warm[:, :64],
                                     start=True, stop=True)

            # run a warm burst during the input-DMA dead window so the qk
            # projection starts with the clock gate already released
            _warm_pe("s", 40)

            # ---- load inputs ----
            # x lands as column blocks (all channel rows of positions
            # [1024*nq, 1024*(nq+1))) so the nq-outer qk projection can start
            # after ~2MB instead of the full 4MB transfer
            xb = [[None] * KC for _ in range(NQ)]
            wqk_sb = []
            for kc in range(KC):
                t = sb.tile([128, 2 * CL], F16, name=f"wqk{kc}", tag=f"wqk{kc}")
                eng = nc.scalar if kc % 2 == 0 else nc.sync
                eng.dma_start(t[:], wqkT.ap()[128 * kc:128 * (kc + 1), :])
                wqk_sb.append(t)
                t = sb.tile([128, 1024], F16, name=f"xb0_{kc}", tag=f"xb0_{kc}")
                eng = nc.sync if kc % 2 == 0 else nc.scalar
                eng.dma_start(t[:], xT.ap()[128 * kc:128 * (kc + 1), 0:1024])
                xb[0][kc] = t
            bqk_sb = []
            for m in range(4):
                t = sb.tile([128, 1], F32, name=f"bqk{m}", tag=f"bqk{m}")
                nc.sync.dma_start(t[:], bqk.ap()[128 * m:128 * (m + 1), :])
                bqk_sb.append(t)
            for kc in range(KC):
                t = sb.tile([128, 1024], F16, name=f"xb1_{kc}", tag=f"xb1_{kc}")
                eng = nc.sync if kc % 2 == 0 else nc.scalar
                eng.dma_start(t[:], xT.ap()[128 * kc:128 * (kc + 1), 1024:2048])
                xb[1][kc] = t
            cos_sb = sb.tile([128, N], F16, name="cos_sb", tag="cos_sb")
            nc.sync.dma_start(cos_sb[:], cos2.ap())
            sin_sb = sb.tile([128, N], F16, name="sin_sb", tag="sin_sb")
            nc.scalar.dma_start(sin_sb[:], sin2s.ap())
            wv_sb = []
            for kc in range(KC):
                t = sb.tile([128, CL], F16, name=f"wv{kc}", tag=f"wv{kc}")
                eng = nc.sync if kc % 2 == 0 else nc.scalar
                eng.dma_start(t[:], wvT.ap()[128 * kc:128 * (kc + 1), :])
                wv_sb.append(t)
            wproj_sb = []
            for p in range(2):
                t = sb.tile([128, C], BF16, name=f"wproj{p}", tag=f"wproj{p}")
                nc.sync.dma_start(t[:], wprojT.ap()[128 * p:128 * (p + 1), :])
                wproj_sb.append(t)

            # ---- qk projection + RoPE ----
            # chunk m rows: m=0:[q_h0,q_h1] m=1:[q_h2,q_h3] m=2:[k_h0,k_h1] m=3:[k_h2,k_h3]
            # Per-head q/k land DUPLICATED into both 64-partition halves of
            # their own [128, N] tile (second copy via SBUF->SBUF DMA on the
            # idle sync/gpsimd queues): the scores matmul then runs as TWO
            # CONCURRENT K=64 row-tiled matmuls (tile_position (0,0)/(64,0)),
            # one per 512-query half -- half the PE time of the zero-padded
            # K=128 form.
            q_dup = [sb.tile([128, N], F16, name=f"qdup{h}", tag=f"qdup{h}")
                     for h in range(4)]
            k_dup = [sb.tile([128, N], F16, name=f"kdup{h}", tag=f"kdup{h}")
                     for h in range(4)]
            swap_mask = [i ^ 1 for i in range(32)]
            qks_t = [sb.tile([128, N], F16, name=f"qks{m}", tag=f"qks{m}")
                     for m in range(4)]
            _dup_eng = [0]

            def rope_slice(m, nq):
                # RoPE on a [128, 1024] column slice of qks[m]:
                #   qk' = qks*cos2 + shift(qks)*sin2s
                # (pair-swap of adjacent partitions via DVE stream shuffle)
                sl = slice(1024 * nq, 1024 * (nq + 1))
                qks = qks_t[m]
                shf = sb.tile([128, 1024], F16, name=f"shf{m}_{nq}", tag="shf",
                              bufs=2)
                nc.vector.stream_shuffle(shf[:], qks[:, sl], swap_mask)
                t2 = sb.tile([128, 1024], F16, name=f"rtmp{m}_{nq}",
                             tag="ropetmp", bufs=2)
                nc.vector.tensor_mul(t2[:], shf[:], sin_sb[:, sl])
                t1 = sb.tile([128, 1024], F16, name=f"rtc{m}_{nq}",
                             tag="ropetc", bufs=2)
                nc.vector.tensor_mul(t1[:], qks[:, sl], cos_sb[:, sl])
                dup = q_dup if m < 2 else k_dup
                lo, hi = dup[2 * (m % 2)], dup[2 * (m % 2) + 1]
                nc.vector.tensor_add(lo[0:64, sl], t1[0:64, :], t2[0:64, :])
                nc.vector.tensor_add(hi[64:128, sl], t1[64:128, :],
                                     t2[64:128, :])
                for t, src, dst in ((lo, slice(0, 64), slice(64, 128)),
                                    (hi, slice(64, 128), slice(0, 64))):
                    eng = nc.sync if _dup_eng[0] % 2 == 0 else nc.gpsimd
                    _dup_eng[0] += 1
                    eng.dma_start(t[dst, sl], t[src, sl])

            # nq-outer so the first matmuls need only the first x column
            # block; 4 m-accumulators fill all 8 PSUM banks per nq.
            # Evict order [2,0,3,1] + selective RoPE so heads 0/1 (k from
            # m=2, q from m=0) are ready as early as possible; the remaining
            # RoPE slices are interleaved into head 0's attention pipeline
            # where the vector engine has slack.
            for nq in range(NQ):
                accs = [ps.tile([128, 1024], F32, name=f"qacc{nq}_{m}",
                                tag="sc" if m < 2 else "oacc", bufs=2)
                        for m in range(4)]
                for kc in range(KC):
                    for m in range(4):
                        for n2 in range(2):
                            nc.tensor.matmul(
                                accs[m][:, 512 * n2:512 * (n2 + 1)],
                                wqk_sb[kc][:, 128 * m:128 * (m + 1)],
                                xb[nq][kc][:, 512 * n2:512 * (n2 + 1)],
                                start=(kc == 0), stop=(kc == KC - 1))
                for m in (2, 0, 3, 1):
                    nc.scalar.activation(
                        qks_t[m][:, 1024 * nq:1024 * (nq + 1)],
                        accs[m][:],
                        mybir.ActivationFunctionType.Identity,
                        bias=bqk_sb[m][:])
                if nq == 0:
                    rope_slice(2, 0)
                    rope_slice(0, 0)
                else:
                    rope_slice(2, 1)
            rope_deferred = [(1, 0), (3, 0), (3, 1), (0, 1), (1, 1)]

            # ---- v projection (natural [j, ch] layout, ones col appended per head) ----
            # j-chunk pairs; pairs 0-1 run before attention, pairs 2-7 are
            # interleaved into head 0's matmul stream (the exp engines set
            # head 0's pace, so the PE slots are free)
            vaug = [None] * NJ

            def vproj_pair(jp):
                jcs = (2 * jp, 2 * jp + 1)
                pvs = [ps.tile([128, CL], F32, name=f"pv{jc}", tag="sc",
                               bufs=2) for jc in jcs]
                for kc in range(KC):
                    for a, jc in enumerate(jcs):
                        nc.tensor.matmul(
                            pvs[a][:],
                            xb[jc // 8][kc][:, 128 * (jc % 8):128 * (jc % 8 + 1)],
                            wv_sb[kc][:],
                            start=(kc == 0), stop=(kc == KC - 1))
                for a, jc in enumerate(jcs):
                    va = sb.tile([128, HL * (D + 1)], BF16, name=f"vaug{jc}",
                                 tag=f"vaug{jc}")
                    nc.vector.memset(va[:, D::D + 1], 1.0)
                    nc.scalar.activation(
                        va.rearrange("p (h e) -> p h e", e=D + 1)[:, :, 0:D],
                        pvs[a].rearrange("p (h e) -> p h e", e=D)[:, :, :],
                        mybir.ActivationFunctionType.Copy)
                    vaug[jc] = va

            for jp in range(NJ // 2):
                vproj_pair(jp)
            for mq in rope_deferred:
                rope_slice(*mq)

            # per-partition bias AP used to shift scores before exp
            eshift = sb.tile([128, 1], F32, name="eshift", tag="eshift")
            nc.vector.memset(eshift[:], SHIFT)
            # K=1 ones row used to broadcast denominators across partitions
            ones64 = sb.tile([1, 64], BF16, name="ones64", tag="ones64")
            nc.vector.memset(ones64[:], 1.0)

            # ---- attention + projection partials, per i-chunk ----
            chunks = [(0, 1024), (1024, 1024)]

            def finalize_head(ih, hl, oacc, o_pair, cw):
                # normalize: o[:, i] / den[i].  Broadcast den across
                # partitions with a K=1 matmul, then reciprocal+mul.
                den = sb.tile([1, cw], BF16, name=f"den{ih}_{hl}",
                              tag="den", bufs=2)
                # vector engine: the scalar queue is the attention j-limiter
                # and an extra ~1.1us ACTIVATE per head makes it slip
                nc.vector.tensor_copy(den[:], oacc[64:65, :])
                rb = ps.tile([64, cw], F32, name=f"rb{ih}_{hl}",
                             tag="oacc", bufs=2)
                for q in range(cw // 512):
                    nc.tensor.matmul(rb[:, 512 * q:512 * (q + 1)],
                                     ones64[:],
                                     den[:, 512 * q:512 * (q + 1)],
                                     start=True, stop=True)
                rr = sb.tile([64, cw], F32, name=f"rr{ih}_{hl}", tag="rr",
                             bufs=2)
                nc.vector.reciprocal_approx_fast(rr[:], rb[:])
                nc.vector.tensor_mul(
                    o_pair[hl // 2][64 * (hl % 2):64 * (hl % 2) + 64, :],
                    oacc[0:64, :], rr[:])

            for ih, (i0, cw) in enumerate(chunks):
                ns = cw // 512
                o_pair = [sb.tile([128, cw], BF16, name=f"opair{ih}_{p}",
                                  tag=f"opair{p}", bufs=2) for p in range(2)]
                if ih > 0:
                    _warm_pe(f"c{ih}", 8)
                pending = None
                pending_o = None
                for hl in range(4):
                    qT = q_dup[hl]
                    kT = k_dup[hl]
                    oacc = ps.tile([65, cw], F32, name=f"oacc{ih}_{hl}",
                                   tag="oacc", bufs=2)
                    exs = []

                    def emit_o(jc, oacc=oacc, exs=exs, hl=hl):
                        for q in range(ns):
                            nc.tensor.matmul(
                                oacc[:, 512 * q:512 * (q + 1)],
                                vaug[jc][:, (D + 1) * hl:(D + 1) * (hl + 1)],
                                exs[jc][:, 512 * q:512 * (q + 1)],
                                start=(jc == 0), stop=(jc == NJ - 1))

                    for jc in range(NJ):
                        sc = ps.tile([128, cw], F32, name=f"sc{ih}_{hl}_{jc}",
                                     tag="sc", bufs=2)
                        # two concurrent K=64 row-tiled matmuls, one per
                        # 512-query half (disjoint row-groups 0-1 / 2-3)
                        for q in range(ns):
                            rp = 64 * (q % 2)
                            nc.tensor.matmul(
                                sc[:, 512 * q:512 * (q + 1)],
                                kT[rp:rp + 64, 128 * jc:128 * (jc + 1)],
                                qT[rp:rp + 64, i0 + 512 * q:i0 + 512 * (q + 1)],
                                start=True, stop=True,
                                tile_position=(rp, 0))
                        ex = sb.tile([128, cw], BF16, name=f"ex{ih}_{hl}_{jc}",
                                     tag="ex", bufs=6)
                        # exp split: scalar engine takes a 576-col stretch
                        # (true exp), vector engine the other 448 (bit-trick
                        # exp; DVE reads PSUM at ~half rate so it gets the
                        # smaller share); sides swap each j to mix the
                        # approx error across queries
                        wa = 608
                        pa = (jc % 2) * (cw - wa)
                        pv = (wa if jc % 2 == 0 else 0)
                        nc.scalar.activation(ex[:, pa:pa + wa],
                                             sc[:, pa:pa + wa],
                                             mybir.ActivationFunctionType.Exp,
                                             scale=SCALE,
                                             bias=eshift[:])
                        nc.vector.tensor_scalar(
                            ex[:, pv:pv + (cw - wa)].bitcast(U16),
                            sc[:, pv:pv + (cw - wa)],
                            EXP_A, EXP_B,
                            mybir.AluOpType.mult, mybir.AluOpType.add)
                        exs.append(ex)
                        # software pipeline: o-matmuls lag TWO j-chunks so
                        # the ~1.2us scores->exp->weights latency stays off
                        # the matmul critical path; the previous head's LAST
                        # two o-matmuls and its normalization chain are both
                        # deferred into this head's pipeline so neither the
                        # matmul nor the exp stream pauses at head boundaries
                        if jc == 1:
                            if pending_o is not None:
                                for t in range(NJ - 2, NJ):
                                    pending_o(t)
                                pending_o = None
                            if pending is not None:
                                finalize_head(*pending)
                                pending = None
                        if jc >= 2:
                            emit_o(jc - 2)
                    pending_o = emit_o
                    pending = (ih, hl, oacc, o_pair, cw)
                # flush the last head's deferred o-matmuls + normalization;
                # a short warm burst bridges the PE through the chain so the
                # projection starts at full rate
                if pending_o is not None:
                    for t in range(NJ - 2, NJ):
                        pending_o(t)
                    pending_o = None
                _warm_pe(f"p{ih}", 12)
                finalize_head(*pending)
                pending = None

                # out-projection partial for this i-chunk, written straight
                # to the output (host sums the 4 per-group partials)
                for mc in range(8):
                    pp = ps.tile([128, cw], F32, name=f"pp{ih}_{mc}",
                                 tag="sc" if mc % 2 == 0 else "oacc",
                                 bufs=2)
                    for n2 in range(ns):
                        isl = slice(512 * n2, 512 * (n2 + 1))
                        for p in range(2):
                            nc.tensor.matmul(
                                pp[:, isl],
                                wproj_sb[p][:, 128 * mc:128 * (mc + 1)],
                                o_pair[p][:, isl],
                                start=(p == 0), stop=(p == 1))
                    po = sb.tile([128, cw], BF16, name=f"po{ih}_{mc}",
                                 tag="po", bufs=4)
                    # alternate evict engine so slots recycle 2x faster
                    if mc % 2 == 0:
                        nc.vector.tensor_copy(po[:], pp[:])
                    else:
                        nc.scalar.activation(
                            po[:], pp[:],
                            mybir.ActivationFunctionType.Copy)
                    eng = nc.sync if mc % 2 == 0 else nc.gpsimd
                    eng.dma_start(
                        outp.ap()[128 * mc:128 * (mc + 1), i0:i0 + cw], po[:])

    nc.compile()
    return nc


def shard_inputs(x, rope, w_qkv, b_qkv, w_proj, b_proj, n_cores=N_CORES):
    """Per-core input maps. Host-side transposes/casts are part of sharding."""
    in_maps = []
    for c in range(n_cores):
        b = (c // 4) % B
        g = c % 4
        heads = range(HL * g, HL * g + HL)

        xTb = np.ascontiguousarray(x[b].T).astype(NPF16)          # [C, N]

        cosT = rope[b].T[:D, :]                                   # [64, N]
        sinT = rope[b].T[D:, :]
        cos2 = np.vstack([cosT, cosT]).astype(NPF16)              # [128, N]
        sgn = np.where(np.arange(128) % 2 == 0, -1.0, 1.0)[:, None]
        sin2s = (np.vstack([sinT, sinT]) * sgn).astype(NPF16)     # [128, N]

        # qk weight rows ordered [q_h0..q_h3, k_h0..k_h3]
        qk_rows = []
        bqk_rows = []
        for h in heads:
            qk_rows.append(w_qkv[D * h:D * (h + 1), :])           # q rows
            bqk_rows.append(b_qkv[D * h:D * (h + 1)])
        for h in heads:
            qk_rows.append(w_qkv[C + D * h:C + D * (h + 1), :])   # k rows
            bqk_rows.append(b_qkv[C + D * h:C + D * (h + 1)])
        wqk = np.vstack(qk_rows)                                  # [512, C]
        wqkT = np.ascontiguousarray(wqk.T).astype(NPF16)          # [C, 512]
        bqk_v = np.concatenate(bqk_rows).astype(np.float32)[:, None]

        h0 = HL * g
        wv = w_qkv[2 * C + D * h0:2 * C + D * h0 + CL, :]          # [256, C]
        wvT = np.ascontiguousarray(wv.T).astype(NPF16)             # [C, 256]

        wp = w_proj[:, D * h0:D * h0 + CL]                         # [C, 256]
        wprojT = np.ascontiguousarray(wp.T).astype(NPBF16)         # [256, C]

        in_maps.append({
            "xT": xTb, "cos2": cos2, "sin2s": sin2s,
            "wqkT": wqkT, "bqk": bqk_v, "wvT": wvT,
            "wprojT": wprojT,
        })
    return in_maps


def assemble(results, b_eff, n_cores=N_CORES):
    out = np.empty((B, N, C), dtype=np.float32)
    for b in range(B):
        acc = np.zeros((C, N), dtype=np.float32)
        for g in range(4):
            acc += results[4 * b + g]["outp"].astype(np.float32)
        out[b] = acc.T + b_eff[None, :]
    return out


_NC_CACHE = {}


def _get_nc():
    if "nc" not in _NC_CACHE:
        _NC_CACHE["nc"] = build_kernel()
    return _NC_CACHE["nc"]


def _run(inputs, trace=False, tmpdir=None):
    nc = _get_nc()
    inputs = {k: np.asarray(v) for k, v in inputs.items()}
    # fold the v-bias through the projection into the output bias (host side)
    b_v = inputs["b_qkv"][2 * C:3 * C]
    b_eff = (inputs["b_proj"] + b_v @ inputs["w_proj"].T).astype(np.float32)
    in_maps = shard_inputs(**inputs)
    res = run_bass_kernel_spmd(nc, in_maps, core_ids=list(range(N_CORES)),
                               trace=trace, tmpdir=tmpdir)
    return assemble(res.results, b_eff), res


def kernel(**inputs):
    out, _ = _run(inputs)
    return out


# revision 46
# speedup vs baseline: 1.0104x; 1.0037x over previous
"""Multi-head attention (B=2, N=2048, C=1024, H=16, D=64) on 8 TRN2 NeuronCores.

Sharding: core c = (batch b = c//4) x (head-group g = c%4 -> heads 4g..4g+3).
Data parallel on B, tensor parallel on heads.  Each core emits its
out-projection PARTIAL [C, N] in bf16; the host sums the 4 partials of each
batch group and adds the (folded) bias.  No on-device collectives.

Softmax exp is split across two engines: the scalar engine computes true
exp on one 512-column half of each [128, 1024] score tile while the vector
engine computes a Schraudolph bit-trick exp (uint16 = round(A*raw + B),
bits reinterpreted as bf16) on the other half; halves swap each j so the
~3% approx error mixes across queries.

Everything on device stays transposed ([channel, position]); the host
pre-transposes inputs and post-transposes the output.
"""

import numpy as np
import ml_dtypes

import concourse.bacc as bacc
import concourse.tile as tile
import concourse.mybir as mybir
from concourse.bass_utils import run_bass_kernel_spmd

B, N, C, H = 2, 2048, 1024, 16
D = C // H          # 64
HL = H // 4         # 4 heads per core
CL = HL * D         # 256 local channels
N_CORES = 8

F32 = mybir.dt.float32
F16 = mybir.dt.float16
BF16 = mybir.dt.bfloat16
U16 = mybir.dt.uint16
NPF16 = np.float16
NPBF16 = ml_dtypes.bfloat16

KC = C // 128       # 8  K-chunks of the input channel dim
NJ = N // 128       # 16 128-row j-chunks
NQ = N // 1024      # 2  1024-wide column blocks for the qk projection

SCALE = float(1.0 / np.sqrt(D))
SHIFT = -16.0       # static exp shift; softmax is shift-invariant
# Schraudolph fake-exp in the bf16 bit domain:
#   u16 = round(EXP_A * raw + EXP_B); bits(u16 << 16) ~ exp(SCALE*raw + SHIFT)
_S = 128.0 / float(np.log(2.0))
EXP_A = float(_S * SCALE)
EXP_B = float(_S * SHIFT + 127.0 * 128.0 - 4.75)


def build_kernel(n_cores=N_CORES):
    nc = bacc.Bacc("TRN2", target_bir_lowering=False, debug=False,
                   num_devices=n_cores)

    xT = nc.declare_dram_parameter("xT", [C, N], F16, isOutput=False)
    cos2 = nc.declare_dram_parameter("cos2", [128, N], F16, isOutput=False)
    sin2s = nc.declare_dram_parameter("sin2s", [128, N], F16, isOutput=False)
    wqkT = nc.declare_dram_parameter("wqkT", [C, 2 * CL], F16, isOutput=False)
    bqk = nc.declare_dram_parameter("bqk", [2 * CL, 1], F32, isOutput=False)
    wvT = nc.declare_dram_parameter("wvT", [C, CL], F16, isOutput=False)
    wprojT = nc.declare_dram_parameter("wprojT", [CL, C], BF16, isOutput=False)
    outp = nc.declare_dram_parameter("outp", [C, N], BF16, isOutput=True)

    with tile.TileContext(nc) as tc:
        with tc.tile_pool(name="sbuf", bufs=1) as sb, \
             tc.tile_pool(name="psum", bufs=1, space="PSUM") as ps:

            # tile for clock-warming matmuls (see _warm_pe)
            warm = sb.tile([128, 128], F16, name="warm", tag="warm")
            nc.vector.memset(warm[:], 0.001)

            def _warm_pe(tag, n):
                # short matmuls alternating two PSUM tiles: keeps the PE's
                # activity monitor busy so the clock gate stays at full rate
                # (~60ns each warm; size n to span the bridge window)
                wps = [ps.tile([128, 64], F32, name=f"warmp{tag}_{a}",
                               tag="sc", bufs=2) for a in range(2)]
                for r in range(n):
                    nc.tensor.matmul(wps[r % 2][:], warm[:], warm[:, :64],
                                     start=True, stop=True)

            # run a warm burst during the input-DMA dead window so the qk
            # projection starts with the clock gate already released
            _warm_pe("s", 40)

            # ---- load inputs ----
            # x lands as column blocks (all channel rows of positions
            # [1024*nq, 1024*(nq+1))) so the nq-outer qk projection can start
            # after ~2MB instead of the full 4MB transfer
            xb = [[None] * KC for _ in range(NQ)]
            wqk_sb = []
            for kc in range(KC):
                t = sb.tile([128, 2 * CL], F16, name=f"wqk{kc}", tag=f"wqk{kc}")
                eng = nc.scalar if kc % 2 == 0 else nc.sync
                eng.dma_start(t[:], wqkT.ap()[128 * kc:128 * (kc + 1), :])
                wqk_sb.append(t)
                t = sb.tile([128, 1024], F16, name=f"xb0_{kc}", tag=f"xb0_{kc}")
                eng = nc.sync if kc % 2 == 0 else nc.scalar
                eng.dma_start(t[:], xT.ap()[128 * kc:128 * (kc + 1), 0:1024])
                xb[0][kc] = t
            bqk_sb = []
            for m in range(4):
                t = sb.tile([128, 1], F32, name=f"bqk{m}", tag=f"bqk{m}")
                nc.sync.dma_start(t[:], bqk.ap()[128 * m:128 * (m + 1), :])
                bqk_sb.append(t)
            for kc in range(KC):
                t = sb.tile([128, 1024], F16, name=f"xb1_{kc}", tag=f"xb1_{kc}")
                eng = nc.sync if kc % 2 == 0 else nc.scalar
                eng.dma_start(t[:], xT.ap()[128 * kc:128 * (kc + 1), 1024:2048])
                xb[1][kc] = t
            cos_sb = sb.tile([128, N], F16, name="cos_sb", tag="cos_sb")
            nc.sync.dma_start(cos_sb[:], cos2.ap())
            sin_sb = sb.tile([128, N], F16, name="sin_sb", tag="sin_sb")
            nc.scalar.dma_start(sin_sb[:], sin2s.ap())
            wv_sb = []
            for kc in range(KC):
                t = sb.tile([128, CL], F16, name=f"wv{kc}", tag=f"wv{kc}")
                eng = nc.sync if kc % 2 == 0 else nc.scalar
                eng.dma_start(t[:], wvT.ap()[128 * kc:128 * (kc + 1), :])
                wv_sb.append(t)
            wproj_sb = []
            for p in range(2):
                t = sb.tile([128, C], BF16, name=f"wproj{p}", tag=f"wproj{p}")
                nc.sync.dma_start(t[:], wprojT.ap()[128 * p:128 * (p + 1), :])
                wproj_sb.append(t)

            # ---- qk projection + RoPE ----
            # chunk m rows: m=0:[q_h0,q_h1] m=1:[q_h2,q_h3] m=2:[k_h0,k_h1] m=3:[k_h2,k_h3]
            # Per-head q/k land DUPLICATED into both 64-partition halves of
            # their own [128, N] tile (second copy via SBUF->SBUF DMA on the
            # idle sync/gpsimd queues): the scores matmul then runs as TWO
            # CONCURRENT K=64 row-tiled matmuls (tile_position (0,0)/(64,0)),
            # one per 512-query half -- half the PE time of the zero-padded
            # K=128 form.
            q_dup = [sb.tile([128, N], F16, name=f"qdup{h}", tag=f"qdup{h}")
                     for h in range(4)]
            k_dup = [sb.tile([128, N], F16, name=f"kdup{h}", tag=f"kdup{h}")
                     for h in range(4)]
            swap_mask = [i ^ 1 for i in range(32)]
            qks_t = [sb.tile([128, N], F16, name=f"qks{m}", tag=f"qks{m}")
                     for m in range(4)]
            _dup_eng = [0]

            def rope_slice(m, nq):
                # RoPE on a [128, 1024] column slice of qks[m]:
                #   qk' = qks*cos2 + shift(qks)*sin2s
                # (pair-swap of adjacent partitions via DVE stream shuffle)
                sl = slice(1024 * nq, 1024 * (nq + 1))
                qks = qks_t[m]
                shf = sb.tile([128, 1024], F16, name=f"shf{m}_{nq}", tag="shf",
                              bufs=2)
                nc.vector.stream_shuffle(shf[:], qks[:, sl], swap_mask)
                t2 = sb.tile([128, 1024], F16, name=f"rtmp{m}_{nq}",
                             tag="ropetmp", bufs=2)
                nc.vector.tensor_mul(t2[:], shf[:], sin_sb[:, sl])
                t1 = sb.tile([128, 1024], F16, name=f"rtc{m}_{nq}",
                             tag="ropetc", bufs=2)
                nc.vector.tensor_mul(t1[:], qks[:, sl], cos_sb[:, sl])
                dup = q_dup if m < 2 else k_dup
                lo, hi = dup[2 * (m % 2)], dup[2 * (m % 2) + 1]
                nc.vector.tensor_add(lo[0:64, sl], t1[0:64, :], t2[0:64, :])
                nc.vector.tensor_add(hi[64:128, sl], t1[64:128, :],
                                     t2[64:128, :])
                for t, src, dst in ((lo, slice(0, 64), slice(64, 128)),
                                    (hi, slice(64, 128), slice(0, 64))):
                    eng = nc.sync if _dup_eng[0] % 2 == 0 else nc.gpsimd
                    _dup_eng[0] += 1
                    eng.dma_start(t[dst, sl], t[src, sl])

            # nq-outer so the first matmuls need only the first x column
            # block; 4 m-accumulators fill all 8 PSUM banks per nq.
            # Evict order [2,0,3,1] + selective RoPE so heads 0/1 (k from
            # m=2, q from m=0) are ready as early as possible; the remaining
            # RoPE slices are interleaved into head 0's attention pipeline
            # where the vector engine has slack.
            for nq in range(NQ):
                accs = [ps.tile([128, 1024], F32, name=f"qacc{nq}_{m}",
                                tag="sc" if m < 2 else "oacc", bufs=2)
                        for m in range(4)]
                for kc in range(KC):
                    for m in range(4):
                        for n2 in range(2):
                            nc.tensor.matmul(
                                accs[m][:, 512 * n2:512 * (n2 + 1)],
                                wqk_sb[kc][:, 128 * m:128 * (m + 1)],
                                xb[nq][kc][:, 512 * n2:512 * (n2 + 1)],
                                start=(kc == 0), stop=(kc == KC - 1))
                for m in (2, 0, 3, 1):
                    nc.scalar.activation(
                        qks_t[m][:, 1024 * nq:1024 * (nq + 1)],
                        accs[m][:],
                        mybir.ActivationFunctionType.Identity,
                        bias=bqk_sb[m][:])
                if nq == 0:
                    rope_slice(2, 0)
                    rope_slice(0, 0)
                else:
                    rope_slice(2, 1)
            rope_deferred = [(1, 0), (3, 0), (3, 1), (0, 1), (1, 1)]

            # ---- v projection (natural [j, ch] layout, ones col appended per head) ----
            # j-chunk pairs; pairs 0-1 run before attention, pairs 2-7 are
            # interleaved into head 0's matmul stream (the exp engines set
            # head 0's pace, so the PE slots are free)
            vaug = [None] * NJ

            def vproj_pair(jp):
                jcs = (2 * jp, 2 * jp + 1)
                pvs = [ps.tile([128, CL], F32, name=f"pv{jc}", tag="sc",
                               bufs=2) for jc in jcs]
                for kc in range(KC):
                    for a, jc in enumerate(jcs):
                        nc.tensor.matmul(
                            pvs[a][:],
                            xb[jc // 8][kc][:, 128 * (jc % 8):128 * (jc % 8 + 1)],
                            wv_sb[kc][:],
                            start=(kc == 0), stop=(kc == KC - 1))
                for a, jc in enumerate(jcs):
                    va = sb.tile([128, HL * (D + 1)], BF16, name=f"vaug{jc}",
                                 tag=f"vaug{jc}")
                    nc.vector.memset(va[:, D::D + 1], 1.0)
                    nc.scalar.activation(
                        va.rearrange("p (h e) -> p h e", e=D + 1)[:, :, 0:D],
                        pvs[a].rearrange("p (h e) -> p h e", e=D)[:, :, :],
                        mybir.ActivationFunctionType.Copy)
                    vaug[jc] = va

            for jp in range(NJ // 2):
                vproj_pair(jp)
            for mq in rope_deferred:
                rope_slice(*mq)

            # per-partition bias AP used to shift scores before exp
            eshift = sb.tile([128, 1], F32, name="eshift", tag="eshift")
            nc.vector.memset(eshift[:], SHIFT)
            # K=1 ones row used to broadcast denominators across partitions
            ones64 = sb.tile([1, 64], BF16, name="ones64", tag="ones64")
            nc.vector.memset(ones64[:], 1.0)

            # ---- attention + projection partials, per i-chunk ----
            chunks = [(0, 1024), (1024, 1024)]

            def finalize_head(ih, hl, oacc, o_pair, cw):
                # normalize: o[:, i] / den[i].  Broadcast den across
                # partitions with a K=1 matmul, then reciprocal+mul.
                den = sb.tile([1, cw], BF16, name=f"den{ih}_{hl}",
                              tag="den", bufs=2)
                # vector engine: the scalar queue is the attention j-limiter
                # and an extra ~1.1us ACTIVATE per head makes it slip
                nc.vector.tensor_copy(den[:], oacc[64:65, :])
                rb = ps.tile([64, cw], F32, name=f"rb{ih}_{hl}",
                             tag="oacc", bufs=2)
                for q in range(cw // 512):
                    nc.tensor.matmul(rb[:, 512 * q:512 * (q + 1)],
                                     ones64[:],
                                     den[:, 512 * q:512 * (q + 1)],
                                     start=True, stop=True)
                rr = sb.tile([64, cw], F32, name=f"rr{ih}_{hl}", tag="rr",
                             bufs=2)
                nc.vector.reciprocal_approx_fast(rr[:], rb[:])
                nc.vector.tensor_mul(
                    o_pair[hl // 2][64 * (hl % 2):64 * (hl % 2) + 64, :],
                    oacc[0:64, :], rr[:])

            for ih, (i0, cw) in enumerate(chunks):
                ns = cw // 512
                o_pair = [sb.tile([128, cw], BF16, name=f"opair{ih}_{p}",
                                  tag=f"opair{p}", bufs=2) for p in range(2)]
                if ih > 0:
                    _warm_pe(f"c{ih}", 8)
                pending = None
                pending_o = None
                for hl in range(4):
                    qT = q_dup[hl]
                    kT = k_dup[hl]
                    oacc = ps.tile([65, cw], F32, name=f"oacc{ih}_{hl}",
                                   tag="oacc", bufs=2)
                    exs = []

                    def emit_o(jc, oacc=oacc, exs=exs, hl=hl):
                        for q in range(ns):
                            nc.tensor.matmul(
                                oacc[:, 512 * q:512 * (q + 1)],
                                vaug[jc][:, (D + 1) * hl:(D + 1) * (hl + 1)],
                                exs[jc][:, 512 * q:512 * (q + 1)],
                                start=(jc == 0), stop=(jc == NJ - 1))

                    for jc in range(NJ):
                        sc = ps.tile([128, cw], F32, name=f"sc{ih}_{hl}_{jc}",
                                     tag="sc", bufs=2)
                        # two concurrent K=64 row-tiled matmuls, one per
                        # 512-query half (disjoint row-groups 0-1 / 2-3)
                        for q in range(ns):
                            rp = 64 * (q % 2)
                            nc.tensor.matmul(
                                sc[:, 512 * q:512 * (q + 1)],
                                kT[rp:rp + 64, 128 * jc:128 * (jc + 1)],
                                qT[rp:rp + 64, i0 + 512 * q:i0 + 512 * (q + 1)],
                                start=True, stop=True,
                                tile_position=(rp, 0))
                        ex = sb.tile([128, cw], BF16, name=f"ex{ih}_{hl}_{jc}",
                                     tag="ex", bufs=6)
                        # exp split: scalar engine takes a 576-col stretch
                        # (true exp), vector engine the other 448 (bit-trick
                        # exp; DVE reads PSUM at ~half rate so it gets the
                        # smaller share); sides swap each j to mix the
                        # approx error across queries
                        wa = 608
                        pa = (jc % 2) * (cw - wa)
                        pv = (wa if jc % 2 == 0 else 0)
                        nc.scalar.activation(ex[:, pa:pa + wa],
                                             sc[:, pa:pa + wa],
                                             mybir.ActivationFunctionType.Exp,
                                             scale=SCALE,
                                             bias=eshift[:])
                        nc.vector.tensor_scalar(
                            ex[:, pv:pv + (cw - wa)].bitcast(U16),
                            sc[:, pv:pv + (cw - wa)],
                            EXP_A, EXP_B,
                            mybir.AluOpType.mult, mybir.AluOpType.add)
                        exs.append(ex)
                        # software pipeline: o-matmuls lag TWO j-chunks so
                        # the ~1.2us scores->exp->weights latency stays off
                        # the matmul critical path; the previous head's LAST
                        # two o-matmuls and its normalization chain are both
                        # deferred into this head's pipeline so neither the
                        # matmul nor the exp stream pauses at head boundaries
                        if jc == 1:
                            if pending_o is not None:
                                for t in range(NJ - 2, NJ):
                                    pending_o(t)
                                pending_o = None
                            if pending is not None:
                                finalize_head(*pending)
                                pending = None
                        if jc >= 2:
                            emit_o(jc - 2)
                    pending_o = emit_o
                    pending = (ih, hl, oacc, o_pair, cw)
                # flush the last head's deferred o-matmuls + normalization;
                # a short warm burst bridges the PE through the chain so the
                # projection starts at full rate
                if pending_o is not None:
                    for t in range(NJ - 2, NJ):
                        pending_o(t)
                    pending_o = None
                _warm_pe(f"p{ih}", 12)
                finalize_head(*pending)
                pending = None

                # out-projection partial for this i-chunk, written straight
                # to the output (host sums the 4 per-group partials)
                for mc in range(8):
                    pp = ps.tile([128, cw], F32, name=f"pp{ih}_{mc}",
                                 tag="sc" if mc % 2 == 0 else "oacc",
                                 bufs=2)
                    for n2 in range(ns):
                        isl = slice(512 * n2, 512 * (n2 + 1))
                        for p in range(2):
                            nc.tensor.matmul(
                                pp[:, isl],
                                wproj_sb[p][:, 128 * mc:128 * (mc + 1)],
                                o_pair[p][:, isl],
                                start=(p == 0), stop=(p == 1))
                    po = sb.tile([128, cw], BF16, name=f"po{ih}_{mc}",
                                 tag="po", bufs=4)
                    # alternate evict engine so slots recycle 2x faster
                    if mc % 2 == 0:
                        nc.vector.tensor_copy(po[:], pp[:])
                    else:
                        nc.scalar.activation(
                            po[:], pp[:],
                            mybir.ActivationFunctionType.Copy)
                    eng = nc.sync if mc % 2 == 0 else nc.gpsimd
                    eng.dma_start(
                        outp.ap()[128 * mc:128 * (mc + 1), i0:i0 + cw], po[:])

    nc.compile()
    return nc


def shard_inputs(x, rope, w_qkv, b_qkv, w_proj, b_proj, n_cores=N_CORES):
    """Per-core input maps. Host-side transposes/casts are part of sharding."""
    in_maps = []
    for c in range(n_cores):
        b = (c // 4) % B
        g = c % 4
        heads = range(HL * g, HL * g + HL)

        xTb = np.ascontiguousarray(x[b].T).astype(NPF16)          # [C, N]

        cosT = rope[b].T[:D, :]                                   # [64, N]
        sinT = rope[b].T[D:, :]
        cos2 = np.vstack([cosT, cosT]).astype(NPF16)              # [128, N]
        sgn = np.where(np.arange(128) % 2 == 0, -1.0, 1.0)[:, None]
        sin2s = (np.vstack([sinT, sinT]) * sgn).astype(NPF16)     # [128, N]

        # qk weight rows ordered [q_h0..q_h3, k_h0..k_h3]
        qk_rows = []
        bqk_rows = []
        for h in heads:
            qk_rows.append(w_qkv[D * h:D * (h + 1), :])           # q rows
            bqk_rows.append(b_qkv[D * h:D * (h + 1)])
        for h in heads:
            qk_rows.append(w_qkv[C + D * h:C + D * (h + 1), :])   # k rows
            bqk_rows.append(b_qkv[C + D * h:C + D * (h + 1)])
        wqk = np.vstack(qk_rows)                                  # [512, C]
        wqkT = np.ascontiguousarray(wqk.T).astype(NPF16)          # [C, 512]
        bqk_v = np.concatenate(bqk_rows).astype(np.float32)[:, None]

        h0 = HL * g
        wv = w_qkv[2 * C + D * h0:2 * C + D * h0 + CL, :]          # [256, C]
        wvT = np.ascontiguousarray(wv.T).astype(NPF16)             # [C, 256]

        wp = w_proj[:, D * h0:D * h0 + CL]                         # [C, 256]
        wprojT = np.ascontiguousarray(wp.T).astype(NPBF16)         # [256, C]

        in_maps.append({
            "xT": xTb, "cos2": cos2, "sin2s": sin2s,
            "wqkT": wqkT, "bqk": bqk_v, "wvT": wvT,
            "wprojT": wprojT,
        })
    return in_maps


def assemble(results, b_eff, n_cores=N_CORES):
    out = np.empty((B, N, C), dtype=np.float32)
    for b in range(B):
        acc = np.zeros((C, N), dtype=np.float32)
        for g in range(4):
            acc += results[4 * b + g]["outp"].astype(np.float32)
        out[b] = acc.T + b_eff[None, :]
    return out


_NC_CACHE = {}


def _get_nc():
    if "nc" not in _NC_CACHE:
        _NC_CACHE["nc"] = build_kernel()
    return _NC_CACHE["nc"]


def _run(inputs, trace=False, tmpdir=None):
    nc = _get_nc()
    inputs = {k: np.asarray(v) for k, v in inputs.items()}
    # fold the v-bias through the projection into the output bias (host side)
    b_v = inputs["b_qkv"][2 * C:3 * C]
    b_eff = (inputs["b_proj"] + b_v @ inputs["w_proj"].T).astype(np.float32)
    in_maps = shard_inputs(**inputs)
    res = run_bass_kernel_spmd(nc, in_maps, core_ids=list(range(N_CORES)),
                               trace=trace, tmpdir=tmpdir)
    return assemble(res.results, b_eff), res


def kernel(**inputs):
    out, _ = _run(inputs)
    return out
